# revision 48
# baseline (speedup 1.0000x reference)
"""DFlashAttention kernel for Trainium2, tensor-parallel across 8 NeuronCores.

Sharding: Megatron-style head parallelism. Core c owns KV head c and Q heads
4c..4c+3 (matches repeat_interleave grouping), i.e. Wq rows [512c, 512c+512),
Wk/Wv rows [128c, 128c+128), Wo columns [512c, 512c+512). Each core computes a
partial output [QL, H] in bf16; the host sums the 8 partials in fp32
(row-parallel Wo).

Engine-level design (v4, tuned against real NTFF hardware traces):
  - Every matmul streams bf16 operands (f32r moving runs at ~1.5 cyc/row on
    real TRN2; fp8 was measured numerically unusable for this data).
  - PSUM is managed as four 4 KB slots ([128, 2, 512] f32 = 2 banks each):
    projections accumulate in packed pairs, attention scores come in
    k-tile pairs so one ACT exp covers [128, 1024] (the ~0.3 us/instr
    sequencer+semaphore tax on ACT was the softmax pacer), PV accumulates
    in bank 0 of a pair tile while the denominator rowsum and the
    reciprocal broadcast recycle bank 1 -- head epilogues allocate no
    extra PSUM and never stall the PE.
  - The denominator is a bf16 binary tree of DVE adds (2x mode) over the
    exp pairs; one ones-stationary matmul does the final cross-partition
    reduce. Epilogues are deferred into the next head's j-loop.
  - The Wo phase of block qb is emitted interleaved into block qb+1's
    score loop (1024-wide moving operands), filling PE slack while ACT
    paces the exps.
  - RoPE rotate-half is a PE permutation matmul; sin sign folded host-side.
  - All DRAM tensors are host-pre-arranged so every DMA is a large
    contiguous-per-partition transfer.
"""

import math
from contextlib import ExitStack

import ml_dtypes
import numpy as np

import concourse.bass as bass
import concourse.bacc as bacc
import concourse.mybir as mybir
import concourse.tile as tile
from concourse.bass_utils import run_bass_kernel_spmd

F32 = mybir.dt.float32
F32R = mybir.dt.float32r
BF16 = mybir.dt.bfloat16
AF = mybir.ActivationFunctionType
ALU = mybir.AluOpType

# Full-problem dims (hardcoded per spec)
B, QL, CTX, H = 1, 2048, 2048, 4096
NH, NKV, HD = 32, 8, 128
NCORES = 8
HPC = NH // NKV  # 4 q-heads per core (one KV head per core)


def build_program(ql=QL, ctx_len=CTX, h=H, trace_sim=False, phases="ABC",
                  body_reps=1):
    """Build the per-core Bass program (SPMD: same program, per-core shards)."""
    s = ql + ctx_len          # total kv length
    et = h // 128             # e-tiles (contraction tiles for projections)
    kt = s // 128             # k-tiles in attention
    QC = 512                  # phase A position-chunk
    nch = ql // QC
    assert ctx_len == ql, "phase A chunking assumes ctx_len == ql"
    QB = 512                  # phase B q-block
    nqb = ql // QB
    scale = 1.0 / math.sqrt(HD)
    DQ = HPC * HD             # 512: per-core q-head dim
    PIECES = 8                # activation DMA pieces per chunk
    EPP = et // PIECES        # e-tiles per piece
    np2 = kt // 2             # k-tile pairs

    nc = bacc.Bacc("TRN2", target_bir_lowering=False, debug=False)

    def din(name, shape, dt_=F32):
        return nc.dram_tensor(name, shape, dt_, kind="ExternalInput").ap()

    hid_r = din("hid_r", [128, nch, et, QC], BF16)    # (p, c, e, q)
    tgt_r = din("tgt_r", [128, nch, et, QC], BF16)
    cosT = din("cosT", [HD, s], BF16)                 # (d, pos)
    sinT = din("sinT", [HD, s], BF16)                 # sign-folded
    wq_r = din("wq_r", [128, et, DQ], BF16)           # (p, e, d)
    wk_r = din("wk_r", [128, et, HD], BF16)
    wv_r = din("wv_r", [128, et, HD], BF16)
    wo_r = din("wo_r", [128, HPC, h], BF16)           # (p, t, o)
    perm_d = din("perm", [128, 128], BF16)            # rotate-half permutation
    ident_d = din("ident", [128, 128], BF16)          # PE transpose identity
    ones_d = din("ones", [128, 128], BF16)            # broadcast stationary
    ones1_d = din("ones1", [128, 1], BF16)            # rowsum stationary
    out_d = nc.dram_tensor("out", [ql, h], BF16, kind="ExternalOutput").ap()

    with tile.TileContext(nc, trace_sim=trace_sim) as tc, ExitStack() as ctx:
        persist = ctx.enter_context(tc.tile_pool(name="persist", bufs=1))
        ps = ctx.enter_context(
            tc.tile_pool(name="ps", bufs=4, space=bass.MemorySpace.PSUM)
        )

        def big(name):
            # one 4 KB PSUM slot = 2 banks = [128, 2, QB] f32
            return ps.tile([128, 2, QB], F32, tag="big", name=name)

        qr_sb = persist.tile([128, HPC, ql], BF16, tag="qr")   # [d, h, q]
        kr_sb = persist.tile([128, s], BF16, tag="kr")         # [d, k]
        v_sb = persist.tile([128, kt, HD], BF16, tag="v")      # [k%128, kt, d]
        perm_sb = persist.tile([128, 128], BF16, tag="perm")
        ident_sb = persist.tile([128, 128], BF16, tag="ident")
        ones_sb = persist.tile([128, 128], BF16, tag="ones")
        ones1_sb = persist.tile([128, 1], BF16, tag="ones1")
        wq_sb = persist.tile([128, et, DQ], BF16, tag="wq")
        wk_sb = persist.tile([128, et, HD], BF16, tag="wk")
        wv_sb = persist.tile([128, et, HD], BF16, tag="wv")
        wo_sb = persist.tile([128, HPC, h], BF16, tag="wo")
        # ALL phase-A inputs ride the single sync queue in exact deadline
        # order (weights for piece p right before piece p's activations,
        # cos/sin as per-chunk slices just before each chunk's epilogue, Wo
        # mid-stream in chunk 2). SDMA engines round-robin between active
        # queues at packet granularity, so a second busy queue would steal
        # bandwidth from the activation stream exactly when it is the
        # critical path; one queue in FIFO order IS the priority schedule.
        nc.scalar.dma_start(perm_sb[:], perm_d[:])
        nc.scalar.dma_start(ident_sb[:], ident_d[:])
        nc.scalar.dma_start(ones_sb[:], ones_d[:])
        nc.scalar.dma_start(ones1_sb[:], ones1_d[:])

        for _rep in range(body_reps):
          # The score/exp emitter is shared between phase B's main pipeline
          # and the A->B bridge: the last chunk's epilogue pre-emits the
          # first two score pairs so ACT starts exp'ing (and the PE pipeline
          # fills) while the epilogue's rope/V-transpose work drains. The ex
          # tiles live in a pool that spans both phases.
          bridge = ctx.enter_context(tc.tile_pool(name=f"bridge{_rep}",
                                                  bufs=1))
          heads = {}

          def ensure_head(qb, hh):
            key = (qb, hh)
            if key not in heads:
                heads[key] = dict(expst=[None] * np2,
                                  levels=[[] for _ in range(6)],
                                  den=[], nden=0, psat2=None)
            return heads[key]

          def head_of(p):
            qb, r = divmod(p, HPC * np2)
            hh, j2 = divmod(r, np2)
            return qb, hh, j2

          def emit_scores(p):
            qb, hh, j2 = head_of(p)
            st = ensure_head(qb, hh)
            qs0 = qb * QB
            # flat [128, 1024] tiles: a 3D [128,2,512] AP makes the ACT
            # (and the exp is the phase-B pacer) split the exp into two
            # ops, paying the ~300ns per-op overhead twice per pair
            pss2 = ps.tile([128, 2 * QB], F32, tag="big", name="pss")
            for u in range(2):
                nc.tensor.matmul(
                    pss2[:, u * QB:u * QB + QB],
                    kr_sb[:, (2 * j2 + u) * 128:(2 * j2 + u) * 128 + 128],
                    qr_sb[:, hh, qs0:qs0 + QB],
                    start=True, stop=True)
            ex = bridge.tile([128, 2 * QB], BF16, tag="expst", bufs=8,
                             name="ex")
            nc.scalar.activation(ex[:, :], pss2[:, :], AF.Exp, scale=scale)
            st["expst"][j2] = ex

          pre_emitted = 0

          # -------- Phase A: projections + RoPE + V transpose --------------
          with tc.tile_pool(name="apool", bufs=1) as apool:
            cos_sb = apool.tile([128, s], BF16, tag="cos")
            sin_sb = apool.tile([128, s], BF16, tag="sin")

            def rope2(src, specs, veng=False):
                # specs: [(u, base, dst)]; one swap-psum slot per pair.
                # veng: route the PSUM->SBUF copies via Vector so they don't
                # sit ahead of phase B's first exps in the ACT queue.
                raws = []
                for u, base, dst in specs:
                    raw = apool.tile([128, QC], BF16, tag="raw", bufs=4,
                                     name=f"raw{u}")
                    if veng:
                        nc.vector.tensor_copy(raw[:], src[:, u, :])
                    else:
                        nc.scalar.copy(raw[:], src[:, u, :])
                    raws.append(raw)
                psw = big("psw")
                for (u, base, dst), raw in zip(specs, raws):
                    nc.tensor.matmul(psw[:, u, :], perm_sb[:], raw[:],
                                     start=True, stop=True)
                for (u, base, dst), raw in zip(specs, raws):
                    t1 = apool.tile([128, QC], BF16, tag="t1", bufs=2)
                    nc.vector.tensor_tensor(
                        t1[:], raw[:], cos_sb[:, base:base + QC], ALU.mult)
                    t2 = apool.tile([128, QC], BF16, tag="t2", bufs=2)
                    nc.vector.tensor_tensor(
                        t2[:], psw[:, u, :], sin_sb[:, base:base + QC],
                        ALU.mult)
                    nc.vector.tensor_tensor(dst, t1[:], t2[:], ALU.add)

            for c in range(nch):
                q0 = c * QC
                kv1 = big("kv1")   # [:,0]=k_noise [:,1]=k_ctx
                kv2 = big("kv2")   # [:,0]=v_noise [:,1]=v_ctx
                q01 = big("q01")
                q23 = big("q23")

                for p in range(PIECES):
                    sl = slice(p * EPP, (p + 1) * EPP)
                    if c == 0:
                        # weights arrive just ahead of the piece that needs
                        # them; piece 0's ride the (otherwise idle) scalar
                        # queue so they land in parallel with the first
                        # activations, the rest share the sync queue
                        weng = nc.scalar if p == 0 else nc.sync
                        weng.dma_start(wk_sb[:, sl, :], wk_r[:, sl, :])
                        weng.dma_start(wv_sb[:, sl, :], wv_r[:, sl, :])
                        weng.dma_start(wq_sb[:, sl, :], wq_r[:, sl, :])
                    hs = apool.tile([128, EPP, QC], BF16, tag="hs", bufs=3)
                    ts_ = apool.tile([128, EPP, QC], BF16, tag="ts", bufs=3)
                    if c == 0 and p == 0:
                        # the very first piece lands while DMA is still slow:
                        # stream it per e-tile so the e=0 matmuls unblock
                        # after ~256KB (Tile tracks subrange deps)
                        for ei in range(EPP):
                            nc.sync.dma_start(hs[:, ei:ei + 1, :],
                                              hid_r[:, c, ei:ei + 1, :])
                            nc.sync.dma_start(ts_[:, ei:ei + 1, :],
                                              tgt_r[:, c, ei:ei + 1, :])
                    else:
                        nc.sync.dma_start(hs[:], hid_r[:, c, sl, :])
                        nc.sync.dma_start(ts_[:], tgt_r[:, c, sl, :])
                    if p == 5:
                        # this chunk's rope constants: needed at the chunk
                        # epilogue, a few pieces from now
                        for base in (q0, ctx_len + q0):
                            nc.sync.dma_start(cos_sb[:, base:base + QC],
                                              cosT[:, base:base + QC])
                            nc.sync.dma_start(sin_sb[:, base:base + QC],
                                              sinT[:, base:base + QC])
                    if c == 2 and p % 2 == 0:
                        # Wo arrives interleaved here: far ahead of its first
                        # use (~300us), far behind the chunk-0/1 crunch
                        nc.sync.dma_start(wo_sb[:, p // 2, :],
                                          wo_r[:, p // 2, :])
                    # K/V first so their PSUM slots release before Q's
                    for ei in range(EPP):
                        e = p * EPP + ei
                        st = dict(start=(e == 0), stop=(e == et - 1))
                        nc.tensor.matmul(kv1[:, 0, :], wk_sb[:, e, :],
                                         hs[:, ei, :], **st)
                        nc.tensor.matmul(kv2[:, 0, :], wv_sb[:, e, :],
                                         hs[:, ei, :], **st)
                        nc.tensor.matmul(kv1[:, 1, :], wk_sb[:, e, :],
                                         ts_[:, ei, :], **st)
                        nc.tensor.matmul(kv2[:, 1, :], wv_sb[:, e, :],
                                         ts_[:, ei, :], **st)
                    for ei in range(EPP):
                        e = p * EPP + ei
                        st = dict(start=(e == 0), stop=(e == et - 1))
                        for hh in range(HPC):
                            dst = q01 if hh < 2 else q23
                            nc.tensor.matmul(
                                dst[:, hh % 2, :],
                                wq_sb[:, e, hh * 128:hh * 128 + 128],
                                hs[:, ei, :], **st)

                # tail: K/V released first so next chunk's K/V matmuls flow
                rope2(kv1, [
                    (0, ctx_len + q0,
                     kr_sb[:, ctx_len + q0:ctx_len + q0 + QC]),
                    (1, q0, kr_sb[:, q0:q0 + QC]),
                ], veng=(c == nch - 1 and "B" in phases))

                def vcopy(veng=False):
                    vds = []
                    for u, kbase in ((1, q0), (0, ctx_len + q0)):
                        vd = apool.tile([128, QC], BF16, tag="vd", bufs=2)
                        if veng:
                            nc.vector.tensor_copy(vd[:], kv2[:, u, :])
                        else:
                            nc.scalar.copy(vd[:], kv2[:, u, :])
                        vds.append((vd, kbase, u))
                    return vds

                def vtrans(vds):
                    pstp = ps.tile([128, 8, 128], BF16, tag="big",
                                   name="pstp")
                    for vd, kbase, u in vds:
                        for i in range(QC // 128):
                            sl2 = (1 - u) * 4 + i
                            nc.tensor.transpose(
                                pstp[:, sl2, :], vd[:, i * 128:i * 128 + 128],
                                ident_sb[:])
                            j = (kbase + i * 128) // 128
                            nc.vector.tensor_copy(v_sb[:, j, :],
                                                  pstp[:, sl2, :])

                last = c == nch - 1 and "B" in phases
                if not last:
                    vtrans(vcopy())
                    rope2(q01, [
                        (0, ctx_len + q0, qr_sb[:, 0, q0:q0 + QC]),
                        (1, ctx_len + q0, qr_sb[:, 1, q0:q0 + QC]),
                    ])
                    rope2(q23, [
                        (0, ctx_len + q0, qr_sb[:, 2, q0:q0 + QC]),
                        (1, ctx_len + q0, qr_sb[:, 3, q0:q0 + QC]),
                    ])
                else:
                    # A->B bridge: kr is fully written once this chunk's kv
                    # rope lands, so the first score pairs are emitted
                    # between the epilogue's rope/V steps -- ACT starts
                    # exp'ing and the PE pipeline fills while the epilogue's
                    # DVE chains drain. V transposes go last (their v_sb
                    # tiles aren't consumed until ~15us into phase B).
                    vds = vcopy(veng=True)  # frees the kv2 PSUM slot early
                    emit_scores(0)
                    rope2(q01, [
                        (0, ctx_len + q0, qr_sb[:, 0, q0:q0 + QC]),
                        (1, ctx_len + q0, qr_sb[:, 1, q0:q0 + QC]),
                    ], veng=True)
                    emit_scores(1)
                    rope2(q23, [
                        (0, ctx_len + q0, qr_sb[:, 2, q0:q0 + QC]),
                        (1, ctx_len + q0, qr_sb[:, 3, q0:q0 + QC]),
                    ], veng=True)
                    emit_scores(2)
                    emit_scores(3)
                    vtrans(vds)
                    # deeper pre-emission: the exps pace the pss-slot
                    # recycling, and the score matmuls spaced through the
                    # epilogue keep HAM from re-throttling the PE clock
                    for pp in range(4, 6):
                        emit_scores(pp)
                    pre_emitted = 6

          # -------- Phase B/C: attention + output projection ----------------
          # One flat pipeline over score pairs p = (qb, hh, j2): scores for
          # pair p+1 are emitted while pair p's exp runs on ACT, the C (Wo)
          # stream of the previous q-block drains between the scores and the
          # PV matmuls (so PV never head-of-line-blocks the PE FIFO waiting
          # on ACT), and each head's softmax epilogue is flushed at the next
          # head's first pair.
          with tc.tile_pool(name="bpool", bufs=1) as bpool:
            NP = (nqb * HPC * np2) if "B" in phases else 0

            def tree_push(st, t, lvl):
                levels = st["levels"]
                levels[lvl].append(t)
                if len(levels[lvl]) == 2:
                    a, b = levels[lvl]
                    levels[lvl] = []
                    o = bpool.tile([128, QB], BF16, tag=f"tr{lvl}", bufs=2,
                                   name=f"tr{lvl}")
                    nc.vector.tensor_tensor(o[:], a[:], b[:], ALU.add)
                    if lvl + 1 == 3:
                        # lvl-3 partials feed the denominator rowsum early
                        # (shortens the head-boundary DVE tail)
                        st["den"].append(o)
                    else:
                        tree_push(st, o, lvl + 1)

            pending = []               # (psat2, acc, qb) per finished head
            ats_by_qb = [[] for _ in range(nqb)]
            pending_c = []
            part0 = {}                 # qb0's first-pass Wo partials

            def make_c_steps(qs0, ats, tset, mode, last=False):
                # mode 'full': contract all heads, copy out, DMA.
                # qb0 has no earlier C stream to fill its PE slack, so its
                # Wo contraction is split: mode 'first' contracts heads
                # {0,1} into SBUF partials as soon as those heads flush
                # (draining during qb0's back half); mode 'second' contracts
                # heads {2,3}, merges with the partial on DVE, and ships.
                steps = []
                state = {}

                def mk_mm(qs, oc2, t, u):
                    def f():
                        if t == tset[0] and u == 0:
                            state[(qs, oc2)] = big("pso")
                        nc.tensor.matmul(
                            state[(qs, oc2)][:, u, :],
                            ats[t][:, qs * 128:qs * 128 + 128],
                            wo_sb[:, t, oc2 * 1024 + u * 512:
                                  oc2 * 1024 + u * 512 + 512],
                            start=(t == tset[0]), stop=(t == tset[-1]))
                    return f

                def mk_evac(qs, oc2):
                    def f():
                        pso2 = state.pop((qs, oc2))
                        c0 = oc2 * 1024
                        # all evacuation copies ride the Vector engine: a
                        # copy in the ACT queue delays the next exp by ~1us
                        # and that stall surfaces as a PE bubble every pair
                        if mode == "first":
                            dst = part0[qs][:, c0:c0 + 1024]
                            nc.vector.tensor_copy(dst, pso2[:, :, :])
                            return
                        orow = state["orow"]
                        if mode == "second":
                            for u in range(2):
                                nc.vector.tensor_tensor(
                                    orow[:, c0 + u * 512:c0 + u * 512 + 512],
                                    pso2[:, u, :],
                                    part0[qs][:, c0 + u * 512:
                                              c0 + u * 512 + 512],
                                    ALU.add)
                        else:
                            nc.vector.tensor_copy(orow[:, c0:c0 + 1024],
                                                  pso2[:, :, :])
                        # ship this 128x1024 chunk now (finer DMAs shrink
                        # the end-of-kernel drain); the final q-block's
                        # chunks alternate two queues -- its ACT queue is
                        # exp-free by then and the drain is bandwidth-bound
                        eng = nc.scalar if (last and oc2 % 2) else nc.sync
                        eng.dma_start(
                            out_d[qs0 + qs * 128:qs0 + qs * 128 + 128,
                                  c0:c0 + 1024],
                            orow[:, c0:c0 + 1024])
                    return f

                def mk_orow(qs):
                    def f():
                        if mode == "first":
                            part0[qs] = bpool.tile([128, h], BF16, tag="oa",
                                                   bufs=4, name=f"oa{qs}")
                        else:
                            state["orow"] = bpool.tile([128, h], BF16,
                                                       tag="orow", bufs=2,
                                                       name=f"orow{qs}")
                    return f

                for qs in range(4):
                    steps.append(mk_orow(qs))
                    for oc2 in range(h // 1024):
                        for u in range(2):
                            for t in tset:
                                steps.append(mk_mm(qs, oc2, t, u))
                        steps.append(mk_evac(qs, oc2))
                return steps

            def drain_c(n):
                while pending_c and n > 0:
                    pending_c[0]()
                    pending_c.pop(0)
                    n -= 1

            def flush_a():
                # part 1: reciprocal of the accumulated denominator (spare
                # bank) + PV evacuation, both on DVE
                psat2, qb = pending[0]
                recip = bpool.tile([1, QB], F32, tag="recip", bufs=2)
                nc.vector.reciprocal_approx_fast(recip[:], psat2[0:1, 1, :])
                recb = bpool.tile([1, QB], BF16, tag="recb", bufs=2)
                nc.vector.tensor_copy(recb[:], recip[:])
                at_raw = bpool.tile([128, QB], BF16, tag="atraw", bufs=2)
                nc.vector.tensor_copy(at_raw[:], psat2[:, 0, :])
                pending[0] = (psat2, qb, recb, at_raw)

            def flush_one():
                # part 2: broadcast 1/den across partitions with a tiny
                # ones-stationary matmul into the spare bank (GpSimd's
                # partition_broadcast pays a multi-us Q7 cold-start), then
                # scale. Runs one pair after flush_a so the PE never waits
                # on the reciprocal.
                psat2, qb, recip, at_raw = pending.pop(0)
                nc.tensor.matmul(psat2[:, 1, :], ones_sb[0:1, :], recip[:],
                                 start=True, stop=True)
                at_sb = bpool.tile([128, QB], BF16, tag="attnT", bufs=9)
                nc.vector.tensor_tensor(at_sb[:], at_raw[:], psat2[:, 1, :],
                                        ALU.mult)
                ats_by_qb[qb].append(at_sb)
                n = len(ats_by_qb[qb])
                if qb == 0 and n == 2:
                    pending_c.extend(
                        make_c_steps(0, ats_by_qb[0], [0, 1], "first"))
                elif n == HPC:
                    if qb == 0:
                        pending_c.extend(
                            make_c_steps(0, ats_by_qb[0], [2, 3], "second"))
                    else:
                        pending_c.extend(
                            make_c_steps(qb * QB, ats_by_qb[qb],
                                         list(range(HPC)), "full",
                                         last=(qb == nqb - 1)))

            # PV consumption lags score emission by two pairs: the exp's
            # ~1.1us latency is then always covered by a full iteration of
            # PE work, so PV never reaches the FIFO head before its ex tile
            # is ready (the data waits in SBUF; PSUM pressure is unchanged).
            next_emit = pre_emitted
            if next_emit == 0 and NP > 0:
                emit_scores(0)
                next_emit = 1
            for it in range(NP + 1):
                if next_emit <= it + 1 and next_emit < NP:
                    emit_scores(next_emit)
                    next_emit += 1
                p = it - 1
                if p < 0:
                    continue
                qb, hh, j2 = head_of(p)
                st = heads[(qb, hh)]
                if j2 == 0:
                    drain_c(2)
                    st["psat2"] = big("psat2")
                else:
                    if j2 == 1 and pending:
                        flush_one()
                    drain_c(3)
                ex = st["expst"][j2]
                psat2 = st["psat2"]
                for u in range(2):
                    j = 2 * j2 + u
                    nc.tensor.matmul(psat2[:, 0, :], v_sb[:, j, :],
                                     ex[:, u * QB:u * QB + QB],
                                     start=(j == 0), stop=(j == kt - 1))
                tp = bpool.tile([128, QB], BF16, tag="tsum", bufs=3,
                                name="tp")
                nc.vector.tensor_tensor(tp[:], ex[:, 0:QB], ex[:, QB:2 * QB],
                                        ALU.add)
                tree_push(st, tp, 0)
                # denominator: accumulate lvl-2 partials into the spare bank
                # as they appear (the final one lands right after tp(15), so
                # the next head's flush never waits a deep DVE tree tail)
                while st["den"]:
                    part = st["den"].pop(0)
                    st["nden"] += 1
                    nc.tensor.matmul(st["psat2"][0:1, 1, :], ones1_sb[:],
                                     part[:], start=(st["nden"] == 1),
                                     stop=(st["nden"] == np2 // 8))
                if j2 == np2 - 1:
                    pending.append((st["psat2"], qb))
                    del heads[(qb, hh)]
                    # reciprocal + PV evacuation start now (their PE inputs
                    # were just emitted), so the psat2 slot recycles before
                    # the next head's PV reaches the FIFO head
                    flush_a()

            while pending:
                flush_one()
            drain_c(len(pending_c))
    nc.compile()
    return nc


def make_in_maps(hidden_states, target_hidden, cos, sin, Wq, Wk, Wv, Wo):
    f32 = np.float32
    bf16 = ml_dtypes.bfloat16
    hidden_states = np.asarray(hidden_states, dtype=f32)
    target_hidden = np.asarray(target_hidden, dtype=f32)
    cos = np.asarray(cos, dtype=f32)
    sin = np.asarray(sin, dtype=f32)
    Wq = np.asarray(Wq, dtype=f32)
    Wk = np.asarray(Wk, dtype=f32)
    Wv = np.asarray(Wv, dtype=f32)
    Wo = np.asarray(Wo, dtype=f32)

    et, nch, QC = H // 128, QL // 512, 512

    def act_r(x):  # [S, H] -> [128, nch, et, QC]: (p, c, e, q)
        xT = x.T  # [H, S]
        return np.ascontiguousarray(
            xT.reshape(et, 128, nch, QC).transpose(1, 2, 0, 3)).astype(bf16)

    hid_r = act_r(hidden_states[0])
    tgt_r = act_r(target_hidden[0])
    cT = np.ascontiguousarray(cos[0].T).astype(bf16)
    sT = np.ascontiguousarray(sin[0].T).copy()
    sT[:64, :] *= -1.0  # fold rotate_half sign: rot(x)*sin == swap(x)*sT
    sT = sT.astype(bf16)

    perm = np.zeros((128, 128), dtype=f32)
    for j in range(128):
        perm[j, (j + 64) % 128] = 1.0
    perm = perm.astype(bf16)

    def w_r(W, d):  # W[d, H] -> [128, et, d]: (p, e, dd)
        return np.ascontiguousarray(
            W.T.reshape(et, 128, d).transpose(1, 0, 2)).astype(bf16)

    in_maps = []
    for c in range(NCORES):
        woT = Wo[:, 512 * c:512 * c + 512].T  # [DQ, H]
        in_maps.append({
            "hid_r": hid_r,
            "tgt_r": tgt_r,
            "cosT": cT,
            "sinT": sT,
            "wq_r": w_r(Wq[512 * c:512 * c + 512, :], 512),
            "wk_r": w_r(Wk[128 * c:128 * c + 128, :], 128),
            "wv_r": w_r(Wv[128 * c:128 * c + 128, :], 128),
            "wo_r": np.ascontiguousarray(
                woT.reshape(HPC, 128, H).transpose(1, 0, 2)).astype(bf16),
            "perm": perm,
            "ident": np.eye(128, dtype=f32).astype(bf16),
            "ones": np.ones((128, 128), dtype=f32).astype(bf16),
            "ones1": np.ones((128, 1), dtype=f32).astype(bf16),
        })
    return in_maps


def combine_outputs(results):
    out = np.zeros((QL, H), dtype=np.float32)
    for r in results:
        out += np.asarray(r["out"], dtype=np.float32)
    return out.reshape(1, QL, H)


_CACHE = {}
LAST_EXEC_NS = None
TRACE = False
TRACE_DIR = None


def kernel(hidden_states, target_hidden, cos, sin, Wq, Wk, Wv, Wo):
    global LAST_EXEC_NS
    if "nc" not in _CACHE:
        _CACHE["nc"] = build_program()
    nc = _CACHE["nc"]
    in_maps = make_in_maps(
        hidden_states, target_hidden, cos, sin, Wq, Wk, Wv, Wo
    )
    kw = {}
    if TRACE and TRACE_DIR:
        kw["tmpdir"] = TRACE_DIR
    res = run_bass_kernel_spmd(
        nc, in_maps, list(range(NCORES)), trace=TRACE, **kw
    )
    LAST_EXEC_NS = res.exec_time_ns
    _CACHE["last_res"] = res
    return combine_outputs(res.results)

